# revision 1
# baseline (speedup 1.0000x reference)
"""EpisodicMemory kernel for Trainium2, 8-core data-parallel.

Reference computation (per batch b, d=32, m=64 memory slots, 2 hops):
    M = vs[b]
    for hop:
        Rh[m,:] = R[b,hop,m] @ h[b,hop,m]                  # batched matvec
        z = [Rh*v, Rh*M, |Rh-v|, |Rh-M|]                   # [m, 4d]
        Z = tanh(z @ W1.T + b1) @ W2.T (+ b2: dropped — softmax-invariant)
        g = softmax(Z over m); o = sum_m ts[b,hop,m] * g[m]
        M = GRUCell(o, M)
    out[b] = M

Sharding: pure data parallel over batch; 128 batches per core.

Numerics: Rs/hs/ts are host-cast to bf16 (DMA bytes halve; DVE runs 2-byte
packed ops at 2x). Einsum reduce over e is an in-place bf16 add-tree on DVE
(TensorReduce gets no 2x mode; the tree does). Softmax and the GRU stay f32.

Per-core layout strategy:
  - einsum Rh: R tiles [128 part=(m,bp), free=(g4,d32,e32)] (b = blk*8+bp*4+g;
    p = m*2+bp, so one 128-partition DMA per block covers 1 MiB at full rate),
    mul (in-place, h broadcast over d) on DVE/Pool alternating, then a 5-level
    in-place add-tree over e on DVE.
  - features built in row layout [128 rows, (g,f,d)] bf16, PE-transposed to
    z^T [feat128, rows] for the MLP matmuls (bf16) on TensorE.
  - softmax/o batched per hop over all 128 batches [128 part=b, 64 m] in f32.
  - GRU in transposed layout [d part, b free], f32; M kept as MT [32,128],
    M_rep rebuilt in bf16 via a DRAM broadcast bounce.
"""

import numpy as np

import concourse.bacc as bacc
import concourse.bass as bass
import concourse.mybir as mybir
import concourse.tile as tile
from concourse.masks import make_identity
from concourse.tile import add_dep_helper

F32 = mybir.dt.float32
BF16 = mybir.dt.bfloat16
AF = mybir.ActivationFunctionType
ALU = mybir.AluOpType
AX = mybir.AxisListType

B, N_HOP, N_MEM, DIM = 1024, 2, 64, 32
N_CORES = 8
BC = B // N_CORES            # 128 batches per core
BB = 8                       # batches per block
NBLK = BC // BB              # 16 blocks
NG = BB // 2                 # 4 b-pair groups per block
ROWS = BB * N_MEM            # 512 rows per block
D4 = 4 * DIM                 # 128 MLP input features


def build_nc(n_iter: int = 1, variant: str = "full") -> bass.Bass:
    """variant: 'full' | 'dma' (loads only) | 'nodma' (R loaded once) |
    'nopool' (all muls on DVE) | 'allpool' (all muls on Pool)."""
    nc = bacc.Bacc("TRN2")

    # Rs/hs arrive host-permuted: [hop, blk, m, bp, g, ...] with b = blk*8+bp*4+g
    Rs_d = nc.dram_tensor(
        "Rs", [N_HOP, NBLK, N_MEM, 2, NG, DIM, DIM], BF16, kind="ExternalInput"
    )
    hs_d = nc.dram_tensor(
        "hs", [N_HOP, N_MEM, 2, NBLK, NG, DIM], BF16, kind="ExternalInput"
    )
    ts_d = nc.dram_tensor("ts", [BC, N_HOP, N_MEM, DIM], BF16, kind="ExternalInput")
    vs_d = nc.dram_tensor("vs", [BC, DIM], F32, kind="ExternalInput")
    W1_d = nc.dram_tensor("W1", [DIM, D4], F32, kind="ExternalInput")
    b1_d = nc.dram_tensor("b1", [DIM], F32, kind="ExternalInput")
    W2_d = nc.dram_tensor("W2", [1, DIM], F32, kind="ExternalInput")
    Wih_d = nc.dram_tensor("W_ih", [N_HOP, 3 * DIM, DIM], F32, kind="ExternalInput")
    Whh_d = nc.dram_tensor("W_hh", [N_HOP, 3 * DIM, DIM], F32, kind="ExternalInput")
    bih_d = nc.dram_tensor("b_ih", [N_HOP, 3 * DIM], F32, kind="ExternalInput")
    bhh_d = nc.dram_tensor("b_hh", [N_HOP, 3 * DIM], F32, kind="ExternalInput")
    out_d = nc.dram_tensor("out", [BC, DIM], F32, kind="ExternalOutput")
    # DRAM bounces for the v/M partition-broadcast (bf16)
    m_flat = nc.dram_tensor("m_flat", [BC, DIM], BF16)
    m_scr2 = nc.dram_tensor("m_scratch2", [2, NBLK, NG, DIM], BF16)
    m_scr3 = nc.dram_tensor("m_scratch3", [128, NBLK * NG * DIM], BF16)
    v_flat = nc.dram_tensor("v_flat", [BC, DIM], BF16)
    v_scr = nc.dram_tensor("v_scratch", [2, NBLK, NG, DIM], BF16)
    v_scr3 = nc.dram_tensor("v_scratch3", [128, NBLK * NG * DIM], BF16)
    # Z bounce, laid out so the gather is contiguous per natural batch index
    z_scr = nc.dram_tensor("z_scratch", [NBLK, 2, NG, N_MEM], F32)

    import contextlib

    with tile.TileContext(nc) as tc:
        with (
            (tc.For_i(0, n_iter, 1) if n_iter > 1 else contextlib.nullcontext()),
            tc.tile_pool(name="consts", bufs=1) as consts,
            tc.tile_pool(name="hop_io", bufs=2) as hop_io,
            tc.tile_pool(name="rpool", bufs=6) as rpool,
            tc.tile_pool(name="fpool", bufs=4) as fpool,
            tc.tile_pool(name="zpool", bufs=4) as zpool,
            tc.tile_pool(name="apool", bufs=4) as apool,
            tc.tile_pool(name="small", bufs=2) as small,
            tc.tile_pool(name="mstate", bufs=2) as mstate,
            tc.tile_pool(name="pp_z", bufs=3, space="PSUM") as pp_z,
            tc.tile_pool(name="pp_1", bufs=2, space="PSUM") as pp_1,
            tc.tile_pool(name="pp_2", bufs=1, space="PSUM") as pp_2,
            tc.tile_pool(name="pp_g", bufs=2, space="PSUM") as pp_g,
        ):
            ident = consts.tile([128, 128], F32)
            make_identity(nc, ident)
            ident16 = consts.tile([128, 128], BF16)
            nc.scalar.copy(out=ident16, in_=ident)

            # ---- weights prep (one-time) ----
            w1_sb = consts.tile([DIM, D4], F32)
            nc.sync.dma_start(out=w1_sb, in_=W1_d[:, :])
            w1t_ps = pp_g.tile([D4, DIM], F32, tag="gpsum")
            nc.tensor.transpose(w1t_ps, w1_sb, ident[:DIM, :DIM])
            W1T = consts.tile([D4, DIM], BF16)
            nc.scalar.copy(out=W1T, in_=w1t_ps)

            W2T_f = consts.tile([DIM, 1], F32)
            nc.sync.dma_start(out=W2T_f, in_=W2_d.rearrange("a b -> b a"))
            W2T = consts.tile([DIM, 1], BF16)
            nc.scalar.copy(out=W2T, in_=W2T_f)
            b1T = consts.tile([DIM, 1], F32)
            nc.sync.dma_start(out=b1T, in_=b1_d[:].unsqueeze(1))

            WihT, WhhT, bsum_rz, bihn_t, bhhn_t = [], [], [], [], []
            for hop in range(N_HOP):
                wih_sb = consts.tile([3 * DIM, DIM], F32, tag="wload", bufs=4)
                nc.sync.dma_start(out=wih_sb, in_=Wih_d[hop])
                wt_ps = pp_g.tile([DIM, 3 * DIM], F32, tag="gpsum")
                nc.tensor.transpose(wt_ps, wih_sb, ident[: 3 * DIM, : 3 * DIM])
                wT = consts.tile([DIM, 3 * DIM], F32, tag=f"wihT{hop}")
                nc.scalar.copy(out=wT, in_=wt_ps)
                WihT.append(wT)

                whh_sb = consts.tile([3 * DIM, DIM], F32, tag="wload", bufs=4)
                nc.sync.dma_start(out=whh_sb, in_=Whh_d[hop])
                wt_ps2 = pp_g.tile([DIM, 3 * DIM], F32, tag="gpsum")
                nc.tensor.transpose(wt_ps2, whh_sb, ident[: 3 * DIM, : 3 * DIM])
                wT2 = consts.tile([DIM, 3 * DIM], F32, tag=f"whhT{hop}")
                nc.scalar.copy(out=wT2, in_=wt_ps2)
                WhhT.append(wT2)

                # per-gate bias tiles, all at base partition 0
                gate_b = []
                for gd, gname in ((bih_d, "ih"), (bhh_d, "hh")):
                    for gate in range(3):
                        bt = consts.tile([DIM, 1], F32, tag=f"b{gname}{hop}{gate}")
                        nc.sync.dma_start(
                            out=bt,
                            in_=gd[hop, gate * DIM : (gate + 1) * DIM].unsqueeze(1),
                        )
                        gate_b.append(bt)
                b_r = consts.tile([DIM, 1], F32, tag=f"b_r{hop}")
                nc.vector.tensor_add(b_r, gate_b[0], gate_b[3])
                b_z = consts.tile([DIM, 1], F32, tag=f"b_z{hop}")
                nc.vector.tensor_add(b_z, gate_b[1], gate_b[4])
                bsum_rz.append((b_r, b_z))
                bihn_t.append(gate_b[2])
                bhhn_t.append(gate_b[5])

            # ---- initial M state ----
            vs_row = consts.tile([BC, DIM], F32)
            nc.sync.dma_start(out=vs_row, in_=vs_d[:, :])
            vst_ps = pp_g.tile([DIM, BC], F32, tag="gpsum")
            nc.tensor.transpose(vst_ps, vs_row, ident)
            vsT = consts.tile([DIM, BC], F32)
            nc.scalar.copy(out=vsT, in_=vst_ps)
            MT = vsT  # current M^T [d, b]

            # v_rep [128 part=(m,bp), (blk,g,d)] bf16: value vs[blk*8+bp*4+g, d].
            # Cast to bf16, stage a (bp, blk, g, d)-permuted copy in DRAM, then
            # replicate per-partition rows in DRAM (free-form APs), then a
            # plain [128, f] load (SBUF DMA APs must be partition-clean on HW).
            vs16 = consts.tile([BC, DIM], BF16)
            nc.scalar.copy(out=vs16, in_=vs_row)
            nc.sync.dma_start(out=v_flat[:, :], in_=vs16)
            nc.sync.dma_start(
                out=v_scr[:, :, :, :],
                in_=v_flat.rearrange("(blk bp g) d -> bp blk g d", bp=2, g=NG),
            )
            nc.sync.dma_start(
                out=v_scr3.rearrange("(m bp) f -> m bp f", bp=2),
                in_=v_scr.rearrange(
                    "bp blk g d -> bp (blk g d)"
                ).partition_broadcast(64),
            )
            v_rep = consts.tile([128, NBLK * NG * DIM], BF16)
            prev_bcast_dma = nc.sync.dma_start(out=v_rep, in_=v_scr3[:, :])

            M_rep = v_rep  # hop 0: M == vs

            for hop in range(N_HOP):
                # per-hop h in einsum layout [(bp,m), (blk,g,e)]
                # h for the whole hop: one contiguous [128, 2048] bf16 load
                h_hop = hop_io.tile([128, NBLK * NG * DIM], BF16, tag="h_hop")
                nc.scalar.dma_start(
                    out=h_hop,
                    in_=hs_d[hop].rearrange("m bp blk g e -> (m bp) (blk g e)"),
                )
                # t natural layout [b, (m,d)]
                t_hop = hop_io.tile([BC, N_MEM * DIM], BF16, tag="t_hop")
                t_dma = nc.scalar.dma_start(
                    out=t_hop, in_=ts_d[:, hop].rearrange("b m d -> b (m d)")
                )
                # lane-ordering: keep the broadcast DMA strictly before t_hop
                add_dep_helper(t_dma.ins, prev_bcast_dma.ins,
                               reason="hwdge lane ordering")

                Z_row = small.tile([BC, N_MEM], F32, tag="Z_row")

                for blk in range(NBLK):
                    if variant == "nodma":
                        if hop == 0 and blk == 0:
                            r_tile = consts.tile(
                                [128, NG * DIM * DIM], BF16, tag="Rconst"
                            )
                            nc.sync.dma_start(
                                out=r_tile,
                                in_=Rs_d[hop, blk].rearrange(
                                    "m bp g d e -> (m bp) (g d e)"
                                ),
                            )
                            r_const = r_tile
                        r_tile = rpool.tile([128, NG * DIM * DIM], BF16, tag="R")
                        nc.vector.tensor_copy(r_tile, r_const)
                    else:
                        r_tile = rpool.tile([128, NG * DIM * DIM], BF16, tag="R")
                        nc.sync.dma_start(
                            out=r_tile,
                            in_=Rs_d[hop, blk].rearrange(
                                "m bp g d e -> (m bp) (g d e)"
                            ),
                        )
                    if variant == "dma":
                        continue
                    # P = R * h (in-place), h broadcast over d
                    r4 = r_tile.rearrange("p (g d e) -> p g d e", g=NG, d=DIM)
                    h_slice = h_hop[
                        :, blk * NG * DIM : (blk + 1) * NG * DIM
                    ].rearrange("p (g e) -> p g e", g=NG)
                    # broadcast-mul on Pool/GPSIMD (engine-time indifferent
                    # to the stride-0 operand) so it overlaps DVE. Asymmetric
                    # split: hop1's einsum shares DVE with hop0's softmax/GRU
                    # tail, so hop1 leans harder on Pool.
                    on_pool = (blk % 3 == 1) if hop == 0 else (blk % 3 != 1)
                    if variant == "nopool":
                        on_pool = False
                    elif variant == "allpool":
                        on_pool = True
                    if on_pool:
                        h_v = h_slice.unsqueeze(2).broadcast_to(
                            (128, NG, DIM, DIM)
                        )
                        nc.gpsimd.tensor_tensor(r4, r4, h_v, op=ALU.mult)
                    else:
                        # DVE: the stride-0 broadcast operand defeats the DVE
                        # 2-byte fast path on HW, so materialize h replicated
                        # over d via log-doubling packed copies, then a fully
                        # packed mul.
                        hr = fpool.tile([128, NG * DIM * DIM], BF16, tag="hrep")
                        h4 = hr.rearrange("p (g d e) -> p g d e", g=NG, d=DIM)
                        nc.vector.tensor_copy(
                            h4[:, :, 0:1, :], h_slice.unsqueeze(2)
                        )
                        for dlo, dhi in ((1, 2), (2, 4), (4, 8), (8, 16), (16, 32)):
                            nc.vector.tensor_copy(
                                h4[:, :, dlo:dhi, :], h4[:, :, 0 : dhi - dlo, :]
                            )
                        nc.vector.tensor_mul(r_tile, r_tile, hr)
                    # Rh[(bp,m), (g,d)] = sum_e P via in-place bf16 add-tree
                    # (TensorReduce gets no 2-byte 2x mode; packed adds do)
                    p3 = r_tile.rearrange("p (gd e) -> p gd e", e=DIM)
                    for half in (16, 8, 4, 2):
                        nc.vector.tensor_add(
                            p3[:, :, :half], p3[:, :, :half], p3[:, :, half : 2 * half]
                        )
                    rh = fpool.tile([128, NG * DIM], BF16, tag="rh")
                    nc.vector.tensor_add(
                        rh.rearrange("p (gd o) -> p gd o", o=1),
                        p3[:, :, 0:1],
                        p3[:, :, 1:2],
                    )
                    # features F [(bp,m), (g, f, d)] bf16
                    f_blk = fpool.tile([128, NG * 4 * DIM], BF16, tag="F")
                    f4 = f_blk.rearrange("p (g f d) -> p g f d", g=NG, f=4)
                    rh3 = rh.rearrange("p (g d) -> p g d", g=NG)
                    vr3 = v_rep[:, blk * NG * DIM : (blk + 1) * NG * DIM].rearrange(
                        "p (g d) -> p g d", g=NG
                    )
                    mr3 = M_rep[:, blk * NG * DIM : (blk + 1) * NG * DIM].rearrange(
                        "p (g d) -> p g d", g=NG
                    )
                    nc.vector.tensor_mul(f4[:, :, 0, :], rh3, vr3)
                    nc.vector.tensor_mul(f4[:, :, 1, :], rh3, mr3)
                    nc.vector.tensor_sub(f4[:, :, 2, :], rh3, vr3)
                    nc.vector.tensor_sub(f4[:, :, 3, :], rh3, mr3)
                    nc.scalar.activation(
                        f4[:, :, 2:4, :], f4[:, :, 2:4, :], AF.Abs
                    )

                    # transpose to z^T [(f,d), (g,bp,m)]
                    zt_ps = pp_z.tile([D4, ROWS], BF16, tag="zt")
                    for g in range(NG):
                        nc.tensor.transpose(
                            zt_ps[:, g * 128 : (g + 1) * 128],
                            f_blk[:, g * 128 : (g + 1) * 128],
                            ident16,
                        )
                    # hop1's MLP burst is Act-paced while DVE sits idle (its
                    # einsum is done before M_rep arrives) — so copy on DVE
                    # there; hop0 overlaps einsum on DVE, keep Act.
                    zt_sb = zpool.tile([D4, ROWS], BF16, tag="zt_sb")
                    if hop == 0:
                        nc.scalar.copy(out=zt_sb, in_=zt_ps)
                    else:
                        nc.vector.tensor_copy(zt_sb, zt_ps)

                    ps1 = pp_1.tile([DIM, ROWS], F32, tag="ps1")
                    nc.tensor.matmul(ps1, lhsT=W1T, rhs=zt_sb, start=True, stop=True)
                    a1 = apool.tile([DIM, ROWS], BF16, tag="a1")
                    nc.scalar.activation(a1, ps1, AF.Tanh, bias=b1T)
                    ps2 = pp_2.tile([1, ROWS], F32, tag="ps2")
                    nc.tensor.matmul(ps2, lhsT=W2T, rhs=a1, start=True, stop=True)
                    z_sb = zpool.tile([1, ROWS], F32, tag="z_sb")
                    if hop == 0:
                        nc.scalar.copy(out=z_sb, in_=ps2)
                    else:
                        nc.vector.tensor_copy(z_sb, ps2)
                    # z_sb free order is (g, m, bp); store as (bp, g, m).
                    # src stays 1-partition (dim0 count 1): split by bp.
                    for bp in range(2):
                        nc.scalar.dma_start(
                            out=z_scr[blk, bp].unsqueeze(0),
                            in_=z_sb.rearrange("o (g m bp) -> o g m bp", g=NG, bp=2)[
                                :, :, :, bp
                            ],
                        )

                # gather Z rows from DRAM: flat (blk,bp,g) == natural b
                nc.scalar.dma_start(
                    out=Z_row,
                    in_=z_scr.rearrange("a b c m -> (a b c) m"),
                )

                # softmax over m, batched over all 128 b. |Z| is small
                # (tanh-bounded second layer), so skip the max-subtract and
                # normalize o AFTER the t-reduction: o = (sum_m t*e) / sum_m e
                e_row = small.tile([BC, N_MEM], F32, tag="e_row")
                nc.scalar.activation(e_row, Z_row, AF.Exp)
                e16 = small.tile([BC, N_MEM], BF16, tag="e16")
                nc.scalar.copy(out=e16, in_=e_row)
                ssum = small.tile([BC, 1], F32, tag="ssum")
                nc.vector.tensor_reduce(out=ssum, in_=e_row, axis=AX.X, op=ALU.add)
                rsum = small.tile([BC, 1], F32, tag="rsum")
                nc.vector.reciprocal(rsum, ssum)

                # o[b,d] = (sum_m t[b,m,d] * e[b,m]) * rsum[b]
                t3 = t_hop.rearrange("b (m d) -> b m d", d=DIM)
                g3 = e16.unsqueeze(2).broadcast_to((BC, N_MEM, DIM))
                nc.vector.tensor_mul(t3, t3, g3)
                # packed add-tree over m instead of the strided TensorReduce
                # (measured ~3x slower than packed adds on HW)
                for mh in (32, 16, 8, 4, 2):
                    nc.vector.tensor_add(
                        t3[:, :mh, :], t3[:, :mh, :], t3[:, mh : 2 * mh, :]
                    )
                o_raw = small.tile([BC, DIM], F32, tag="o_raw")
                nc.vector.tensor_add(
                    o_raw.unsqueeze(1), t3[:, 0:1, :], t3[:, 1:2, :]
                )
                o_row = small.tile([BC, DIM], F32, tag="o_row")
                nc.vector.tensor_scalar_mul(o_row, o_raw, rsum)

                # GRU (transposed layout [*, b], f32)
                ot_ps = pp_g.tile([DIM, BC], F32, tag="gpsum")
                nc.tensor.transpose(ot_ps, o_row, ident)
                oT = small.tile([DIM, BC], F32, tag="oT")
                nc.scalar.copy(out=oT, in_=ot_ps)

                # per-gate matmuls so every gate tile sits at base partition 0
                def gate_pair(g):
                    gi = pp_g.tile([DIM, BC], F32, tag="gpsum")
                    nc.tensor.matmul(
                        gi,
                        lhsT=WihT[hop][:, g * DIM : (g + 1) * DIM],
                        rhs=oT,
                        start=True,
                        stop=True,
                    )
                    gh = pp_g.tile([DIM, BC], F32, tag="gpsum")
                    nc.tensor.matmul(
                        gh,
                        lhsT=WhhT[hop][:, g * DIM : (g + 1) * DIM],
                        rhs=MT,
                        start=True,
                        stop=True,
                    )
                    return gi, gh

                # r,z gates: sigmoid(gi + gh + b_ih + b_hh)
                rz_t = []
                for g in range(2):
                    gi, gh = gate_pair(g)
                    gb = small.tile([DIM, BC], F32, tag=f"g{g}b")
                    nc.scalar.activation(gb, gi, AF.Identity, bias=bsum_rz[hop][g])
                    nc.vector.tensor_add(gb, gb, gh)
                    gt = small.tile([DIM, BC], F32, tag=f"gate{g}")
                    nc.scalar.activation(gt, gb, AF.Sigmoid)
                    rz_t.append(gt)
                r_t, z_t = rz_t

                # n = tanh(gi_n + b_ih_n + r * (gh_n + b_hh_n))
                gi_n, gh_n = gate_pair(2)
                ghn = small.tile([DIM, BC], F32, tag="ghn")
                nc.scalar.activation(ghn, gh_n, AF.Identity, bias=bhhn_t[hop])
                gin = small.tile([DIM, BC], F32, tag="gin")
                nc.scalar.activation(gin, gi_n, AF.Identity, bias=bihn_t[hop])
                n1 = small.tile([DIM, BC], F32, tag="n1")
                nc.vector.tensor_mul(n1, r_t, ghn)
                nc.vector.tensor_add(n1, n1, gin)
                n_t = small.tile([DIM, BC], F32, tag="n_t")
                nc.scalar.activation(n_t, n1, AF.Tanh)

                # M' = n + z * (M - n)
                MT_new = mstate.tile([DIM, BC], F32, tag="MT")
                nc.vector.tensor_sub(MT_new, MT, n_t)
                nc.vector.tensor_mul(MT_new, MT_new, z_t)
                nc.vector.tensor_add(MT_new, MT_new, n_t)
                MT = MT_new

                if hop < N_HOP - 1:
                    # rebuild M_rep (bf16) via DRAM bounce
                    mrow_ps = pp_g.tile([BC, DIM], F32, tag="gpsum")
                    nc.tensor.transpose(mrow_ps, MT, ident[:DIM, :DIM])
                    M16 = mstate.tile([BC, DIM], BF16, tag="M16")
                    nc.scalar.copy(out=M16, in_=mrow_ps)
                    nc.scalar.dma_start(out=m_flat[:, :], in_=M16)
                    nc.sync.dma_start(
                        out=m_scr2[:, :, :, :],
                        in_=m_flat.rearrange(
                            "(blk bp g) d -> bp blk g d", bp=2, g=NG
                        ),
                    )
                    M_rep_new = mstate.tile(
                        [128, NBLK * NG * DIM], BF16, tag="M_rep", bufs=1
                    )
                    nc.sync.dma_start(
                        out=m_scr3.rearrange("(m bp) f -> m bp f", bp=2),
                        in_=m_scr2.rearrange(
                            "bp blk g d -> bp (blk g d)"
                        ).partition_broadcast(64),
                    )
                    prev_bcast_dma = nc.sync.dma_start(
                        out=M_rep_new, in_=m_scr3[:, :]
                    )
                    M_rep = M_rep_new
                else:
                    # row-major M only needed for the final output
                    mrow_ps = pp_g.tile([BC, DIM], F32, tag="gpsum")
                    nc.tensor.transpose(mrow_ps, MT, ident[:DIM, :DIM])
                    M_row = mstate.tile([BC, DIM], F32, tag="M_row")
                    nc.scalar.copy(out=M_row, in_=mrow_ps)
                    nc.sync.dma_start(out=out_d[:, :], in_=M_row)

    nc.compile()
    return nc


_NC_CACHE = None


def _get_nc():
    global _NC_CACHE
    if _NC_CACHE is None:
        _NC_CACHE = build_nc()
    return _NC_CACHE


def _bf16(x):
    import ml_dtypes

    return np.asarray(x).astype(ml_dtypes.bfloat16)


def permute_local(x):
    """[BC, N_HOP, m, ...] -> [N_HOP, NBLK, m, 2, NG, ...] with b = blk*8+bp*4+g."""
    tail = x.shape[2:]
    y = x.reshape(NBLK, 2, NG, N_HOP, *tail)
    order = (3, 0, 4, 1, 2) + tuple(range(5, y.ndim))
    return np.ascontiguousarray(y.transpose(order))


def permute_h(x):
    """hs [BC, N_HOP, m, e] -> [N_HOP, m, 2, NBLK, NG, e]."""
    y = x.reshape(NBLK, 2, NG, N_HOP, N_MEM, DIM)
    return np.ascontiguousarray(y.transpose(3, 4, 1, 0, 2, 5))


def make_in_maps(hs, Rs, ts, vs, W1, b1, W2, W_ih, W_hh, b_ih, b_hh):
    in_maps = []
    for c in range(N_CORES):
        sl = slice(c * BC, (c + 1) * BC)
        in_maps.append(
            {
                "Rs": permute_local(_bf16(Rs[sl])),
                "hs": permute_h(_bf16(hs[sl])),
                "ts": _bf16(ts[sl]),
                "vs": np.ascontiguousarray(vs[sl]),
                "W1": np.ascontiguousarray(W1),
                "b1": np.ascontiguousarray(b1),
                "W2": np.ascontiguousarray(W2),
                "W_ih": np.ascontiguousarray(W_ih),
                "W_hh": np.ascontiguousarray(W_hh),
                "b_ih": np.ascontiguousarray(b_ih),
                "b_hh": np.ascontiguousarray(b_hh),
            }
        )
    return in_maps


def kernel(hs, Rs, ts, vs, W1, b1, W2, b2, W_ih, W_hh, b_ih, b_hh):
    from concourse.bass_utils import run_bass_kernel_spmd

    nc = _get_nc()
    in_maps = make_in_maps(hs, Rs, ts, vs, W1, b1, W2, W_ih, W_hh, b_ih, b_hh)
    res = run_bass_kernel_spmd(nc, in_maps, list(range(N_CORES)))
    return np.concatenate([r["out"] for r in res.results], axis=0)



# revision 19
# speedup vs baseline: 1.1184x; 1.1184x over previous
"""EpisodicMemory kernel for Trainium2, 8-core data-parallel. v2.

Reference computation (per batch b, d=32, m=64 memory slots, 2 hops):
    M = vs[b]
    for hop:
        Rh[m,:] = R[b,hop,m] @ h[b,hop,m]                  # batched matvec
        z = [Rh*v, Rh*M, |Rh-v|, |Rh-M|]                   # [m, 4d]
        Z = tanh(z @ W1.T + b1) @ W2.T (+ b2: dropped — softmax-invariant)
        g = softmax(Z over m); o = sum_m ts[b,hop,m] * g[m]
        M = GRUCell(o, M)
    out[b] = M

Sharding: pure data parallel over batch; 128 batches per core.

v2 design vs v1 (396 us):
  - Rs is host-cast to fp8 e4m3 (end-to-end rel err ~4e-3, gate is 2e-2).
    DMA floor drops from ~127 us (bf16) to ~67 us.
  - The einsum e-reduction moves from a DVE add-tree to the TensorEngine:
    partition layout p=(q32, e4) -- 32 batches x 4 e-values.  P = R*h
    (bf16) is contracted with ONE static block-diagonal-ones lhsT
    [128,32]; 8 e-groups accumulate into PSUM via start/stop.  Four
    super-blocks of 32 batches stack into a [128, 512] PSUM tile using
    32-aligned tile positions, so Rh comes out as [b=128, (m16, d32)]
    chunks -- feature-ready, full partition utilization.
  - The R*h multiply is split DVE (with packed log-doubling h-replication
    over d; h_rep copies are fully packed so they hit the DVE fast path)
    and Pool (gpsimd broadcast-mul, stride-0 tolerant).  One DVE pre-add
    per e-group pair halves the PE streaming.
  - v_rep / M_rep (v for features) are [b, (m16, d)] tiles built with a
    few packed SBUF copies -- the v1 DRAM broadcast bounce is gone.
  - MLP/softmax/GRU tail is v1's (transpose to z^T, two matmuls, DRAM
    Z-bounce gather, max-free softmax, transposed-layout GRU).
"""

import numpy as np

import concourse.bacc as bacc
import concourse.bass as bass
import concourse.mybir as mybir
import concourse.tile as tile
from concourse.masks import make_identity

F32 = mybir.dt.float32
BF16 = mybir.dt.bfloat16
FP8 = mybir.dt.float8e4
AF = mybir.ActivationFunctionType
ALU = mybir.AluOpType
AX = mybir.AxisListType

B, N_HOP, N_MEM, DIM = 1024, 2, 64, 32
N_CORES = 8
BC = B // N_CORES            # 128 batches per core
NSB = 4                      # super-blocks per core
QB = 32                      # batches per super-block (partition q-dim)
EV = 4                       # e-values per partition group
NEG = DIM // EV              # 8 e-groups
MC = 16                      # m per PE output chunk
NCH = N_MEM // MC            # 4 chunks
D4 = 4 * DIM                 # 128 MLP input features

# e-groups with (eg + S) % 2 == 0 take the DVE mul path (R in bf16 with a
# packed half-width h_rep); the others take the Pool broadcast-mul path
# (R in fp8).  h_rep width: h replicated over d up to HREP_W, then the mul
# runs in 2048/HREP_W packed sub-muls reusing the same h_rep.
HREP_W = 512


def build_nc(n_iter: int = 1, variant: str = "full") -> bass.Bass:
    """variant: 'full' | 'dma' (loads only)"""
    nc = bacc.Bacc("TRN2")

    # host-permuted layouts (b = S*32 + q within a core):
    #   Rs_v[hop, S, k, (q,e4), (d,m)]  bf16, e-groups eg = 2k + S%2
    #   Rs_p[hop, S, k, (q,e4), (d,m)]  fp8,  e-groups eg = 2k + 1 - S%2
    #   hs[hop, S, (q,e4), (eg,m)]      bf16
    Rsv_d = nc.dram_tensor(
        "Rs_v", [N_HOP, NSB, NEG // 2, 128, DIM * N_MEM], BF16,
        kind="ExternalInput",
    )
    Rsp_d = nc.dram_tensor(
        "Rs_p", [N_HOP, NSB, NEG // 2, 128, DIM * N_MEM], FP8,
        kind="ExternalInput",
    )
    hs_d = nc.dram_tensor(
        "hs", [N_HOP, NSB, 128, NEG * N_MEM], BF16, kind="ExternalInput"
    )
    ts_d = nc.dram_tensor("ts", [BC, N_HOP, N_MEM, DIM], BF16, kind="ExternalInput")
    vs_d = nc.dram_tensor("vs", [BC, DIM], F32, kind="ExternalInput")
    W1_d = nc.dram_tensor("W1", [DIM, D4], F32, kind="ExternalInput")
    b1_d = nc.dram_tensor("b1", [DIM], F32, kind="ExternalInput")
    W2_d = nc.dram_tensor("W2", [1, DIM], F32, kind="ExternalInput")
    Wih_d = nc.dram_tensor("W_ih", [N_HOP, 3 * DIM, DIM], F32, kind="ExternalInput")
    Whh_d = nc.dram_tensor("W_hh", [N_HOP, 3 * DIM, DIM], F32, kind="ExternalInput")
    bih_d = nc.dram_tensor("b_ih", [N_HOP, 3 * DIM], F32, kind="ExternalInput")
    bhh_d = nc.dram_tensor("b_hh", [N_HOP, 3 * DIM], F32, kind="ExternalInput")
    out_d = nc.dram_tensor("out", [BC, DIM], F32, kind="ExternalOutput")
    # Z bounce: [m-group, m4, b] f32; gather back as [b, m]
    z_scr = nc.dram_tensor("z_scratch", [MC, EV, BC], F32)

    import contextlib

    with tile.TileContext(nc) as tc:
        with (
            (tc.For_i(0, n_iter, 1) if n_iter > 1 else contextlib.nullcontext()),
            tc.tile_pool(name="consts", bufs=1) as consts,
            tc.tile_pool(name="hop_io", bufs=2) as hop_io,
            tc.tile_pool(name="rpool", bufs=12) as rpool,
            tc.tile_pool(name="ppool", bufs=2) as ppool,
            tc.tile_pool(name="fpool", bufs=2) as fpool,
            tc.tile_pool(name="zpool", bufs=2) as zpool,
            tc.tile_pool(name="apool", bufs=2) as apool,
            tc.tile_pool(name="small", bufs=2) as small,
            tc.tile_pool(name="mstate", bufs=2) as mstate,
            tc.tile_pool(name="pp_rh", bufs=1, space="PSUM") as pp_rh,
            tc.tile_pool(name="pp_zt", bufs=1, space="PSUM") as pp_zt,
            tc.tile_pool(name="pp_m", bufs=2, space="PSUM") as pp_m,
            tc.tile_pool(name="pp_2", bufs=1, space="PSUM") as pp_2,
        ):
            ident = consts.tile([128, 128], F32)
            make_identity(nc, ident)
            ident16 = consts.tile([128, 128], BF16)
            nc.scalar.copy(out=ident16, in_=ident)

            # block-diag ones [p=(q,e4), q]: 1 iff p//4 == q.  PSUM matmul
            # outputs may only start at partition 0/32/64, so super-blocks
            # write 64-row halves: lo has the diag in cols 0-31 (S even),
            # hi in cols 32-63 (S odd); the other half-block's rows get +0.
            a2 = ident.rearrange("p (q two) -> p q two", two=2)
            t64 = consts.tile([128, 64], F32)
            nc.vector.tensor_add(t64, a2[:, :, 0], a2[:, :, 1])
            b2v = t64.rearrange("p (q two) -> p q two", two=2)
            t32 = consts.tile([128, 32], F32)
            nc.vector.tensor_add(t32, b2v[:, :, 0], b2v[:, :, 1])
            ones_lo = consts.tile([128, 64], BF16)
            nc.vector.memset(ones_lo, 0)
            nc.scalar.copy(out=ones_lo[:, 0:32], in_=t32)
            ones_hi = consts.tile([128, 64], BF16)
            nc.vector.memset(ones_hi, 0)
            nc.scalar.copy(out=ones_hi[:, 32:64], in_=t32)

            # ---- weights prep (one-time) ----
            w1_sb = consts.tile([DIM, D4], F32)
            nc.sync.dma_start(out=w1_sb, in_=W1_d[:, :])
            w1t_ps = pp_m.tile([D4, DIM], F32, tag="ps1")
            nc.tensor.transpose(w1t_ps, w1_sb, ident[:DIM, :DIM])
            W1T = consts.tile([D4, DIM], BF16)
            nc.scalar.copy(out=W1T, in_=w1t_ps)

            W2T_f = consts.tile([DIM, 1], F32)
            nc.sync.dma_start(out=W2T_f, in_=W2_d.rearrange("a b -> b a"))
            W2T = consts.tile([DIM, 1], BF16)
            nc.scalar.copy(out=W2T, in_=W2T_f)
            b1T = consts.tile([DIM, 1], F32)
            nc.sync.dma_start(out=b1T, in_=b1_d[:].unsqueeze(1))

            WihT, WhhT, bsum_rz, bihn_t, bhhn_t = [], [], [], [], []
            for hop in range(N_HOP):
                wih_sb = consts.tile([3 * DIM, DIM], F32, tag="wload", bufs=4)
                nc.sync.dma_start(out=wih_sb, in_=Wih_d[hop])
                wt_ps = pp_m.tile([DIM, 3 * DIM], F32, tag="ps1")
                nc.tensor.transpose(wt_ps, wih_sb, ident[: 3 * DIM, : 3 * DIM])
                wT = consts.tile([DIM, 3 * DIM], F32, tag=f"wihT{hop}")
                nc.scalar.copy(out=wT, in_=wt_ps)
                WihT.append(wT)

                whh_sb = consts.tile([3 * DIM, DIM], F32, tag="wload", bufs=4)
                nc.sync.dma_start(out=whh_sb, in_=Whh_d[hop])
                wt_ps2 = pp_m.tile([DIM, 3 * DIM], F32, tag="ps1")
                nc.tensor.transpose(wt_ps2, whh_sb, ident[: 3 * DIM, : 3 * DIM])
                wT2 = consts.tile([DIM, 3 * DIM], F32, tag=f"whhT{hop}")
                nc.scalar.copy(out=wT2, in_=wt_ps2)
                WhhT.append(wT2)

                gate_b = []
                for gd, gname in ((bih_d, "ih"), (bhh_d, "hh")):
                    for gate in range(3):
                        bt = consts.tile([DIM, 1], F32, tag=f"b{gname}{hop}{gate}")
                        nc.sync.dma_start(
                            out=bt,
                            in_=gd[hop, gate * DIM : (gate + 1) * DIM].unsqueeze(1),
                        )
                        gate_b.append(bt)
                b_r = consts.tile([DIM, 1], F32, tag=f"b_r{hop}")
                nc.vector.tensor_add(b_r, gate_b[0], gate_b[3])
                b_z = consts.tile([DIM, 1], F32, tag=f"b_z{hop}")
                nc.vector.tensor_add(b_z, gate_b[1], gate_b[4])
                bsum_rz.append((b_r, b_z))
                bihn_t.append(gate_b[2])
                bhhn_t.append(gate_b[5])

            # ---- initial M state ----
            vs_row = consts.tile([BC, DIM], F32)
            nc.sync.dma_start(out=vs_row, in_=vs_d[:, :])
            vst_ps = pp_m.tile([DIM, BC], F32, tag="ps1")
            nc.tensor.transpose(vst_ps, vs_row, ident)
            vsT = consts.tile([DIM, BC], F32)
            nc.scalar.copy(out=vsT, in_=vst_ps)
            MT = vsT  # current M^T [d, b]

            # v_rep [b, (m16, d)] bf16 via packed log-doubling
            v_rep = consts.tile([BC, MC * DIM], BF16)
            nc.vector.tensor_copy(v_rep[:, 0:DIM], vs_row)
            w = DIM
            while w < MC * DIM:
                nc.vector.tensor_copy(v_rep[:, w : 2 * w], v_rep[:, 0:w])
                w *= 2

            M_rep = v_rep  # hop 0: M == vs

            for hop in range(N_HOP):
                # h for the whole hop+sblk: [p=(q,e4), (eg, m)]
                h_sb = []
                for S in range(NSB):
                    h_t = hop_io.tile([128, NEG * N_MEM], BF16, tag="h", bufs=4)
                    nc.scalar.dma_start(out=h_t, in_=hs_d[hop, S])
                    h_sb.append(h_t)
                t_hop = hop_io.tile([BC, N_MEM * DIM], BF16, tag="t_hop")
                nc.scalar.dma_start(
                    out=t_hop, in_=ts_d[:, hop].rearrange("b m d -> b (m d)")
                )

                # Rh accumulators: [b=128, (m16, d32)] f32, one per m-chunk
                rh_ps = [
                    pp_rh.tile([128, MC * DIM], F32, tag=f"rh{c}", name=f"rh{c}")
                    for c in range(NCH)
                ]

                # ---- einsum: Rh = sum_e R*h via PE block-diag reduce ----
                for S in range(NSB):
                    half = (S // 2) * 64
                    ones_bd = ones_lo if S % 2 == 0 else ones_hi
                    for eg in range(NEG):
                        on_dve = (eg + S) % 2 == 0
                        k = eg // 2
                        if on_dve:
                            r_t = rpool.tile(
                                [128, DIM * N_MEM], BF16, tag="Rv"
                            )
                            nc.sync.dma_start(out=r_t, in_=Rsv_d[hop, S, k])
                        else:
                            r_t = rpool.tile(
                                [128, DIM * N_MEM], FP8, tag="Rp"
                            )
                            nc.sync.dma_start(out=r_t, in_=Rsp_d[hop, S, k])
                        if variant == "dma":
                            continue
                        P_t = ppool.tile([128, DIM * N_MEM], BF16, tag="P")
                        h_sl = h_sb[S][:, eg * N_MEM : (eg + 1) * N_MEM]
                        if on_dve:
                            # packed half-width h_rep, reused by sub-muls
                            hr = fpool.tile([128, HREP_W], BF16, tag="hrep")
                            nc.vector.tensor_copy(hr[:, 0:N_MEM], h_sl)
                            w = N_MEM
                            while w < HREP_W:
                                nc.vector.tensor_copy(
                                    hr[:, w : 2 * w], hr[:, 0:w]
                                )
                                w *= 2
                            for k2 in range(0, DIM * N_MEM, HREP_W):
                                nc.vector.tensor_mul(
                                    P_t[:, k2 : k2 + HREP_W],
                                    r_t[:, k2 : k2 + HREP_W],
                                    hr,
                                )
                        else:
                            h_b = h_sl.unsqueeze(1).broadcast_to(
                                (128, DIM, N_MEM)
                            )
                            nc.gpsimd.tensor_tensor(
                                P_t.rearrange("p (d m) -> p d m", d=DIM),
                                r_t.rearrange("p (d m) -> p d m", d=DIM),
                                h_b,
                                op=ALU.mult,
                            )
                        Pm = P_t.rearrange("p (d m) -> p m d", d=DIM)
                        for c in range(NCH):
                            nc.tensor.matmul(
                                rh_ps[c][half : half + 64, :],
                                lhsT=ones_bd,
                                rhs=Pm[:, c * MC : (c + 1) * MC, :],
                                start=(S % 2 == 0 and eg == 0),
                                stop=(S % 2 == 1 and eg == NEG - 1),
                            )

                if variant == "dma":
                    continue

                # ---- features z = [Rh*v, Rh*M, |Rh-v|, |Rh-M|] ----
                z_hop = zpool.tile([BC, N_MEM * 4 * DIM], BF16, tag="z")
                z4 = z_hop.rearrange("b (m f d) -> b m f d", f=4, d=DIM)
                vr3 = v_rep.rearrange("b (m d) -> b m d", d=DIM)
                mr3 = M_rep.rearrange("b (m d) -> b m d", d=DIM)
                for c in range(NCH):
                    mc = slice(c * MC, (c + 1) * MC)
                    rh_sb = fpool.tile([BC, MC * DIM], BF16, tag="rh_sb")
                    # Pool cannot read PSUM on HW; split these on Act/DVE
                    if c % 2 == 0:
                        nc.scalar.copy(out=rh_sb, in_=rh_ps[c])
                    else:
                        nc.vector.tensor_copy(rh_sb, rh_ps[c])
                    rh3 = rh_sb.rearrange("b (m d) -> b m d", d=DIM)
                    nc.vector.tensor_mul(z4[:, mc, 0, :], rh3, vr3)
                    nc.vector.tensor_mul(z4[:, mc, 1, :], rh3, mr3)
                    nc.gpsimd.tensor_tensor(z4[:, mc, 2, :], rh3, vr3, op=ALU.subtract)
                    nc.gpsimd.tensor_tensor(z4[:, mc, 3, :], rh3, mr3, op=ALU.subtract)
                    nc.scalar.activation(
                        z4[:, mc, 2:4, :], z4[:, mc, 2:4, :], AF.Abs
                    )

                # ---- MLP per m4-group: transpose + 2 matmuls + Z bounce ----
                zf = z_hop.rearrange("b (m fd) -> b m fd", fd=4 * DIM)
                for g in range(MC):
                    zt_ps = pp_zt.tile([D4, EV * BC], BF16, tag="zt")
                    for j in range(EV):
                        nc.tensor.transpose(
                            zt_ps[:, j * BC : (j + 1) * BC],
                            zf[:, g * EV + j, :],
                            ident16,
                        )
                    zt_sb = zpool.tile([D4, EV * BC], BF16, tag="zt_sb", bufs=3)
                    if g % 2 == 0:
                        nc.scalar.copy(out=zt_sb, in_=zt_ps)
                    else:
                        nc.vector.tensor_copy(zt_sb, zt_ps)
                    ps1 = pp_m.tile([DIM, EV * BC], F32, tag="ps1")
                    nc.tensor.matmul(ps1, lhsT=W1T, rhs=zt_sb, start=True, stop=True)
                    a1 = apool.tile([DIM, EV * BC], BF16, tag="a1")
                    nc.scalar.activation(a1, ps1, AF.Tanh, bias=b1T)
                    ps2 = pp_2.tile([1, EV * BC], F32, tag="ps2")
                    nc.tensor.matmul(ps2, lhsT=W2T, rhs=a1, start=True, stop=True)
                    z_sb = zpool.tile([1, EV * BC], F32, tag="z_sb", bufs=3)
                    if g % 2 == 0:
                        nc.scalar.copy(out=z_sb, in_=ps2)
                    else:
                        nc.vector.tensor_copy(z_sb, ps2)
                    nc.sync.dma_start(
                        out=z_scr[g].rearrange("m4 b -> (m4 b)").unsqueeze(0),
                        in_=z_sb,
                    )

                # gather Z rows from DRAM: [b, m] with m = (g, m4)
                Z_row = small.tile([BC, N_MEM], F32, tag="Z_row")
                nc.scalar.dma_start(
                    out=Z_row, in_=z_scr.rearrange("g m4 b -> b (g m4)")
                )

                # softmax over m (skip max-subtract; |Z| tanh-bounded),
                # normalize o after the t-reduction
                e_row = small.tile([BC, N_MEM], F32, tag="e_row")
                nc.scalar.activation(e_row, Z_row, AF.Exp)
                e16 = small.tile([BC, N_MEM], BF16, tag="e16")
                nc.scalar.copy(out=e16, in_=e_row)
                ssum = small.tile([BC, 1], F32, tag="ssum")
                nc.vector.tensor_reduce(out=ssum, in_=e_row, axis=AX.X, op=ALU.add)
                rsum = small.tile([BC, 1], F32, tag="rsum")
                nc.vector.reciprocal(rsum, ssum)

                # o[b,d] = (sum_m t[b,m,d] * e[b,m]) * rsum[b]
                t3 = t_hop.rearrange("b (m d) -> b m d", d=DIM)
                g3 = e16.unsqueeze(2).broadcast_to((BC, N_MEM, DIM))
                nc.gpsimd.tensor_tensor(t3, t3, g3, op=ALU.mult)
                for mh in (32, 16, 8, 4, 2):
                    nc.vector.tensor_add(
                        t3[:, :mh, :], t3[:, :mh, :], t3[:, mh : 2 * mh, :]
                    )
                o_raw = small.tile([BC, DIM], F32, tag="o_raw")
                nc.vector.tensor_add(
                    o_raw.unsqueeze(1), t3[:, 0:1, :], t3[:, 1:2, :]
                )
                o_row = small.tile([BC, DIM], F32, tag="o_row")
                nc.vector.tensor_scalar_mul(o_row, o_raw, rsum)

                # ---- GRU (transposed layout [*, b], f32) ----
                ot_ps = pp_m.tile([DIM, BC], F32, tag="ps1")
                nc.tensor.transpose(ot_ps, o_row, ident)
                oT = small.tile([DIM, BC], F32, tag="oT")
                nc.scalar.copy(out=oT, in_=ot_ps)

                def gate_pair(g):
                    gi = pp_m.tile([DIM, BC], F32, tag="ps1")
                    nc.tensor.matmul(
                        gi,
                        lhsT=WihT[hop][:, g * DIM : (g + 1) * DIM],
                        rhs=oT,
                        start=True,
                        stop=True,
                    )
                    gh = pp_m.tile([DIM, BC], F32, tag="ps1")
                    nc.tensor.matmul(
                        gh,
                        lhsT=WhhT[hop][:, g * DIM : (g + 1) * DIM],
                        rhs=MT,
                        start=True,
                        stop=True,
                    )
                    return gi, gh

                rz_t = []
                for g in range(2):
                    gi, gh = gate_pair(g)
                    gb = small.tile([DIM, BC], F32, tag=f"g{g}b")
                    nc.scalar.activation(gb, gi, AF.Identity, bias=bsum_rz[hop][g])
                    nc.vector.tensor_add(gb, gb, gh)
                    gt = small.tile([DIM, BC], F32, tag=f"gate{g}")
                    nc.scalar.activation(gt, gb, AF.Sigmoid)
                    rz_t.append(gt)
                r_t, z_t = rz_t

                gi_n, gh_n = gate_pair(2)
                ghn = small.tile([DIM, BC], F32, tag="ghn")
                nc.scalar.activation(ghn, gh_n, AF.Identity, bias=bhhn_t[hop])
                gin = small.tile([DIM, BC], F32, tag="gin")
                nc.scalar.activation(gin, gi_n, AF.Identity, bias=bihn_t[hop])
                n1 = small.tile([DIM, BC], F32, tag="n1")
                nc.vector.tensor_mul(n1, r_t, ghn)
                nc.vector.tensor_add(n1, n1, gin)
                n_t = small.tile([DIM, BC], F32, tag="n_t")
                nc.scalar.activation(n_t, n1, AF.Tanh)

                # M' = n + z * (M - n)
                MT_new = mstate.tile([DIM, BC], F32, tag="MT")
                nc.vector.tensor_sub(MT_new, MT, n_t)
                nc.vector.tensor_mul(MT_new, MT_new, z_t)
                nc.vector.tensor_add(MT_new, MT_new, n_t)
                MT = MT_new

                mrow_ps = pp_m.tile([BC, DIM], F32, tag="ps1")
                nc.tensor.transpose(mrow_ps, MT, ident[:DIM, :DIM])
                if hop < N_HOP - 1:
                    # M_rep [b, (m16, d)] bf16 via packed log-doubling
                    M_rep_new = mstate.tile([BC, MC * DIM], BF16, tag="M_rep")
                    nc.scalar.copy(out=M_rep_new[:, 0:DIM], in_=mrow_ps)
                    w = DIM
                    while w < MC * DIM:
                        nc.vector.tensor_copy(
                            M_rep_new[:, w : 2 * w], M_rep_new[:, 0:w]
                        )
                        w *= 2
                    M_rep = M_rep_new
                else:
                    M_row = mstate.tile([BC, DIM], F32, tag="M_row")
                    nc.scalar.copy(out=M_row, in_=mrow_ps)
                    nc.sync.dma_start(out=out_d[:, :], in_=M_row)

    nc.compile()
    return nc


_NC_CACHE = None


def _get_nc():
    global _NC_CACHE
    if _NC_CACHE is None:
        _NC_CACHE = build_nc()
    return _NC_CACHE


def _bf16(x):
    import ml_dtypes

    return np.asarray(x).astype(ml_dtypes.bfloat16)


def _fp8(x):
    import ml_dtypes

    return np.asarray(x).astype(ml_dtypes.float8_e4m3)


def permute_R(x):
    """Rs [BC, N_HOP, m, d, e] -> (Rs_v bf16, Rs_p fp8), each
    [hop, S, k, (q,e4), (d,m)]: eg = 2k + S%2 on the v side."""
    y = x.reshape(NSB, QB, N_HOP, N_MEM, DIM, NEG, EV)
    # [S, q, hop, m, d, eg, e4] -> [hop, S, eg, q, e4, d, m]
    y = y.transpose(2, 0, 5, 1, 6, 4, 3).reshape(
        N_HOP, NSB, NEG, 128, DIM * N_MEM
    )
    idx_v = np.empty((NSB, NEG // 2), dtype=np.int64)
    idx_p = np.empty((NSB, NEG // 2), dtype=np.int64)
    for S in range(NSB):
        for k in range(NEG // 2):
            idx_v[S, k] = 2 * k + S % 2
            idx_p[S, k] = 2 * k + 1 - S % 2
    rv = np.stack(
        [y[:, S, idx_v[S]] for S in range(NSB)], axis=1
    )
    rp = np.stack(
        [y[:, S, idx_p[S]] for S in range(NSB)], axis=1
    )
    return np.ascontiguousarray(_bf16(rv)), np.ascontiguousarray(_fp8(rp))


def permute_h(x):
    """hs [BC, N_HOP, m, e] -> [hop, S, (q,e4), (eg,m)] bf16."""
    y = x.reshape(NSB, QB, N_HOP, N_MEM, NEG, EV)
    # [S, q, hop, m, eg, e4] -> [hop, S, q, e4, eg, m]
    y = y.transpose(2, 0, 1, 5, 4, 3)
    return np.ascontiguousarray(y.reshape(N_HOP, NSB, 128, NEG * N_MEM))


def make_in_maps(hs, Rs, ts, vs, W1, b1, W2, W_ih, W_hh, b_ih, b_hh):
    in_maps = []
    for c in range(N_CORES):
        sl = slice(c * BC, (c + 1) * BC)
        rv, rp = permute_R(Rs[sl])
        in_maps.append(
            {
                "Rs_v": rv,
                "Rs_p": rp,
                "hs": permute_h(_bf16(hs[sl])),
                "ts": _bf16(ts[sl]),
                "vs": np.ascontiguousarray(vs[sl]),
                "W1": np.ascontiguousarray(W1),
                "b1": np.ascontiguousarray(b1),
                "W2": np.ascontiguousarray(W2),
                "W_ih": np.ascontiguousarray(W_ih),
                "W_hh": np.ascontiguousarray(W_hh),
                "b_ih": np.ascontiguousarray(b_ih),
                "b_hh": np.ascontiguousarray(b_hh),
            }
        )
    return in_maps


def kernel(hs, Rs, ts, vs, W1, b1, W2, b2, W_ih, W_hh, b_ih, b_hh):
    from concourse.bass_utils import run_bass_kernel_spmd

    nc = _get_nc()
    in_maps = make_in_maps(hs, Rs, ts, vs, W1, b1, W2, W_ih, W_hh, b_ih, b_hh)
    res = run_bass_kernel_spmd(nc, in_maps, list(range(N_CORES)))
    return np.concatenate([r["out"] for r in res.results], axis=0)


# revision 28
# speedup vs baseline: 1.2336x; 1.1030x over previous
"""EpisodicMemory kernel for Trainium2, 8-core data-parallel. v2.

Reference computation (per batch b, d=32, m=64 memory slots, 2 hops):
    M = vs[b]
    for hop:
        Rh[m,:] = R[b,hop,m] @ h[b,hop,m]                  # batched matvec
        z = [Rh*v, Rh*M, |Rh-v|, |Rh-M|]                   # [m, 4d]
        Z = tanh(z @ W1.T + b1) @ W2.T (+ b2: dropped — softmax-invariant)
        g = softmax(Z over m); o = sum_m ts[b,hop,m] * g[m]
        M = GRUCell(o, M)
    out[b] = M

Sharding: pure data parallel over batch; 128 batches per core.

v2 design vs v1 (396 us):
  - Rs is host-cast to fp8 e4m3 (end-to-end rel err ~4e-3, gate is 2e-2).
    DMA floor drops from ~127 us (bf16) to ~67 us.
  - The einsum e-reduction moves from a DVE add-tree to the TensorEngine:
    partition layout p=(q32, e4) -- 32 batches x 4 e-values.  P = R*h
    (bf16) is contracted with ONE static block-diagonal-ones lhsT
    [128,32]; 8 e-groups accumulate into PSUM via start/stop.  Four
    super-blocks of 32 batches stack into a [128, 512] PSUM tile using
    32-aligned tile positions, so Rh comes out as [b=128, (m16, d32)]
    chunks -- feature-ready, full partition utilization.
  - The R*h multiply is split DVE (with packed log-doubling h-replication
    over d; h_rep copies are fully packed so they hit the DVE fast path)
    and Pool (gpsimd broadcast-mul, stride-0 tolerant).  One DVE pre-add
    per e-group pair halves the PE streaming.
  - v_rep / M_rep (v for features) are [b, (m16, d)] tiles built with a
    few packed SBUF copies -- the v1 DRAM broadcast bounce is gone.
  - MLP/softmax/GRU tail is v1's (transpose to z^T, two matmuls, DRAM
    Z-bounce gather, max-free softmax, transposed-layout GRU).
"""

import numpy as np

import concourse.bacc as bacc
import concourse.bass as bass
import concourse.mybir as mybir
import concourse.tile as tile
from concourse.masks import make_identity

F32 = mybir.dt.float32
BF16 = mybir.dt.bfloat16
FP8 = mybir.dt.float8e4
AF = mybir.ActivationFunctionType
ALU = mybir.AluOpType
AX = mybir.AxisListType

B, N_HOP, N_MEM, DIM = 1024, 2, 64, 32
N_CORES = 8
BC = B // N_CORES            # 128 batches per core
NSB = 4                      # super-blocks per core
QB = 32                      # batches per super-block (partition q-dim)
EV = 4                       # e-values per partition group
NEG = DIM // EV              # 8 e-groups
MC = 16                      # m per PE output chunk
NCH = N_MEM // MC            # 4 chunks
D4 = 4 * DIM                 # 128 MLP input features

# e-groups with (eg + S) % 2 == 0 take the DVE mul path (R in bf16 with a
# packed half-width h_rep); the others take the Pool broadcast-mul path
# (R in fp8).  h_rep width: h replicated over d up to HREP_W, then the mul
# runs in 2048/HREP_W packed sub-muls reusing the same h_rep.
HREP_W = 512


def build_nc(n_iter: int = 1, variant: str = "full") -> bass.Bass:
    """variant: 'full' | 'dma' (loads only)"""
    nc = bacc.Bacc("TRN2")

    # host-permuted layouts (b = S*32 + q within a core):
    #   Rs_v[hop, S, k, (q,e4), (d,m)]  bf16, e-groups eg = 2k + S%2
    #   Rs_p[hop, S, k, (q,e4), (d,m)]  fp8,  e-groups eg = 2k + 1 - S%2
    #   hs[hop, S, (q,e4), (eg,m)]      bf16
    Rsv_d = nc.dram_tensor(
        "Rs_v", [N_HOP, NSB, NEG // 2, 128, DIM * N_MEM], BF16,
        kind="ExternalInput",
    )
    Rsp_d = nc.dram_tensor(
        "Rs_p", [N_HOP, NSB, NEG // 2, 128, DIM * N_MEM], FP8,
        kind="ExternalInput",
    )
    hs_d = nc.dram_tensor(
        "hs", [N_HOP, NSB, 128, NEG * N_MEM], BF16, kind="ExternalInput"
    )
    ts_d = nc.dram_tensor("ts", [BC, N_HOP, N_MEM, DIM], BF16, kind="ExternalInput")
    vs_d = nc.dram_tensor("vs", [BC, DIM], F32, kind="ExternalInput")
    W1_d = nc.dram_tensor("W1", [DIM, D4], F32, kind="ExternalInput")
    b1_d = nc.dram_tensor("b1", [DIM], F32, kind="ExternalInput")
    W2_d = nc.dram_tensor("W2", [1, DIM], F32, kind="ExternalInput")
    Wih_d = nc.dram_tensor("W_ih", [N_HOP, 3 * DIM, DIM], F32, kind="ExternalInput")
    Whh_d = nc.dram_tensor("W_hh", [N_HOP, 3 * DIM, DIM], F32, kind="ExternalInput")
    bih_d = nc.dram_tensor("b_ih", [N_HOP, 3 * DIM], F32, kind="ExternalInput")
    bhh_d = nc.dram_tensor("b_hh", [N_HOP, 3 * DIM], F32, kind="ExternalInput")
    out_d = nc.dram_tensor("out", [BC, DIM], F32, kind="ExternalOutput")

    import contextlib

    with tile.TileContext(nc) as tc:
        with (
            (tc.For_i(0, n_iter, 1) if n_iter > 1 else contextlib.nullcontext()),
            tc.tile_pool(name="consts", bufs=1) as consts,
            tc.tile_pool(name="hop_io", bufs=2) as hop_io,
            tc.tile_pool(name="rpool", bufs=12) as rpool,
            tc.tile_pool(name="ppool", bufs=2) as ppool,
            tc.tile_pool(name="fpool", bufs=2) as fpool,
            tc.tile_pool(name="zpool", bufs=2) as zpool,
            tc.tile_pool(name="apool", bufs=2) as apool,
            tc.tile_pool(name="small", bufs=2) as small,
            tc.tile_pool(name="mstate", bufs=2) as mstate,
            tc.tile_pool(name="pp_rh", bufs=1, space="PSUM") as pp_rh,
            tc.tile_pool(name="pp_zt", bufs=1, space="PSUM") as pp_zt,
            tc.tile_pool(name="pp_m", bufs=2, space="PSUM") as pp_m,
            tc.tile_pool(name="pp_2", bufs=1, space="PSUM") as pp_2,
        ):
            ident = consts.tile([128, 128], F32)
            make_identity(nc, ident)
            ident16 = consts.tile([128, 128], BF16)
            nc.scalar.copy(out=ident16, in_=ident)

            # block-diag ones [p=(q,e4), q]: 1 iff p//4 == q.  PSUM matmul
            # outputs may only start at partition 0/32/64, so super-blocks
            # write 64-row halves: lo has the diag in cols 0-31 (S even),
            # hi in cols 32-63 (S odd); the other half-block's rows get +0.
            a2 = ident.rearrange("p (q two) -> p q two", two=2)
            t64 = consts.tile([128, 64], F32)
            nc.vector.tensor_add(t64, a2[:, :, 0], a2[:, :, 1])
            b2v = t64.rearrange("p (q two) -> p q two", two=2)
            t32 = consts.tile([128, 32], F32)
            nc.vector.tensor_add(t32, b2v[:, :, 0], b2v[:, :, 1])
            ones_lo = consts.tile([128, 64], BF16)
            nc.vector.memset(ones_lo, 0)
            nc.scalar.copy(out=ones_lo[:, 0:32], in_=t32)
            ones_hi = consts.tile([128, 64], BF16)
            nc.vector.memset(ones_hi, 0)
            nc.scalar.copy(out=ones_hi[:, 32:64], in_=t32)

            # ---- weights prep (one-time) ----
            w1_sb = consts.tile([DIM, D4], F32)
            nc.sync.dma_start(out=w1_sb, in_=W1_d[:, :])
            w1t_ps = pp_m.tile([D4, DIM], F32, tag="ps1")
            nc.tensor.transpose(w1t_ps, w1_sb, ident[:DIM, :DIM])
            W1T = consts.tile([D4, DIM], BF16)
            nc.scalar.copy(out=W1T, in_=w1t_ps)

            W2T_f = consts.tile([DIM, 1], F32)
            nc.sync.dma_start(out=W2T_f, in_=W2_d.rearrange("a b -> b a"))
            # block-diag W2^T [128, 4]: col j = W2^T at partitions 32j..
            w2bd_f = consts.tile([128, EV], F32)
            nc.vector.memset(w2bd_f, 0)
            for j in range(EV):
                nc.scalar.copy(
                    out=w2bd_f[j * DIM : (j + 1) * DIM, j : j + 1], in_=W2T_f
                )
            W2BD = consts.tile([128, EV], BF16)
            nc.scalar.copy(out=W2BD, in_=w2bd_f)
            b1T = consts.tile([DIM, 1], F32)
            nc.sync.dma_start(out=b1T, in_=b1_d[:].unsqueeze(1))

            WihT, WhhT, bsum_rz, bihn_t, bhhn_t = [], [], [], [], []
            for hop in range(N_HOP):
                wih_sb = consts.tile([3 * DIM, DIM], F32, tag="wload", bufs=4)
                nc.sync.dma_start(out=wih_sb, in_=Wih_d[hop])
                wt_ps = pp_m.tile([DIM, 3 * DIM], F32, tag="ps1")
                nc.tensor.transpose(wt_ps, wih_sb, ident[: 3 * DIM, : 3 * DIM])
                wT = consts.tile([DIM, 3 * DIM], F32, tag=f"wihT{hop}")
                nc.scalar.copy(out=wT, in_=wt_ps)
                WihT.append(wT)

                whh_sb = consts.tile([3 * DIM, DIM], F32, tag="wload", bufs=4)
                nc.sync.dma_start(out=whh_sb, in_=Whh_d[hop])
                wt_ps2 = pp_m.tile([DIM, 3 * DIM], F32, tag="ps1")
                nc.tensor.transpose(wt_ps2, whh_sb, ident[: 3 * DIM, : 3 * DIM])
                wT2 = consts.tile([DIM, 3 * DIM], F32, tag=f"whhT{hop}")
                nc.scalar.copy(out=wT2, in_=wt_ps2)
                WhhT.append(wT2)

                gate_b = []
                for gd, gname in ((bih_d, "ih"), (bhh_d, "hh")):
                    for gate in range(3):
                        bt = consts.tile([DIM, 1], F32, tag=f"b{gname}{hop}{gate}")
                        nc.sync.dma_start(
                            out=bt,
                            in_=gd[hop, gate * DIM : (gate + 1) * DIM].unsqueeze(1),
                        )
                        gate_b.append(bt)
                b_r = consts.tile([DIM, 1], F32, tag=f"b_r{hop}")
                nc.vector.tensor_add(b_r, gate_b[0], gate_b[3])
                b_z = consts.tile([DIM, 1], F32, tag=f"b_z{hop}")
                nc.vector.tensor_add(b_z, gate_b[1], gate_b[4])
                bsum_rz.append((b_r, b_z))
                bihn_t.append(gate_b[2])
                bhhn_t.append(gate_b[5])

            # ---- initial M state ----
            vs_row = consts.tile([BC, DIM], F32)
            nc.sync.dma_start(out=vs_row, in_=vs_d[:, :])
            vst_ps = pp_m.tile([DIM, BC], F32, tag="ps1")
            nc.tensor.transpose(vst_ps, vs_row, ident)
            vsT = consts.tile([DIM, BC], F32)
            nc.scalar.copy(out=vsT, in_=vst_ps)
            MT = vsT  # current M^T [d, b]

            # v_rep [b, (m16, d)] bf16 via packed log-doubling
            v_rep = consts.tile([BC, MC * DIM], BF16)
            nc.vector.tensor_copy(v_rep[:, 0:DIM], vs_row)
            w = DIM
            while w < MC * DIM:
                nc.vector.tensor_copy(v_rep[:, w : 2 * w], v_rep[:, 0:w])
                w *= 2

            M_rep = v_rep  # hop 0: M == vs

            for hop in range(N_HOP):
                # h for the whole hop+sblk: [p=(q,e4), (eg, m)]
                h_sb = []
                for S in range(NSB):
                    h_t = hop_io.tile([128, NEG * N_MEM], BF16, tag="h", bufs=4)
                    nc.scalar.dma_start(out=h_t, in_=hs_d[hop, S])
                    h_sb.append(h_t)
                t_hop = hop_io.tile([BC, N_MEM * DIM], BF16, tag="t_hop")
                nc.scalar.dma_start(
                    out=t_hop, in_=ts_d[:, hop].rearrange("b m d -> b (m d)")
                )

                # Rh accumulators: [b=128, (m16, d32)] f32, one per m-chunk
                rh_ps = [
                    pp_rh.tile([128, MC * DIM], F32, tag=f"rh{c}", name=f"rh{c}")
                    for c in range(NCH)
                ]

                # ---- einsum: Rh = sum_e R*h via PE block-diag reduce ----
                for S in range(NSB):
                    half = (S // 2) * 64
                    ones_bd = ones_lo if S % 2 == 0 else ones_hi
                    for eg in range(NEG):
                        on_dve = (eg + S) % 2 == 0
                        k = eg // 2
                        if on_dve:
                            r_t = rpool.tile(
                                [128, DIM * N_MEM], BF16, tag="Rv"
                            )
                            nc.sync.dma_start(out=r_t, in_=Rsv_d[hop, S, k])
                        else:
                            r_t = rpool.tile(
                                [128, DIM * N_MEM], FP8, tag="Rp"
                            )
                            nc.sync.dma_start(out=r_t, in_=Rsp_d[hop, S, k])
                        if variant == "dma":
                            continue
                        P_t = ppool.tile([128, DIM * N_MEM], BF16, tag="P")
                        h_sl = h_sb[S][:, eg * N_MEM : (eg + 1) * N_MEM]
                        if on_dve:
                            # packed half-width h_rep, reused by sub-muls
                            hr = fpool.tile([128, HREP_W], BF16, tag="hrep")
                            nc.vector.tensor_copy(hr[:, 0:N_MEM], h_sl)
                            w = N_MEM
                            while w < HREP_W:
                                nc.vector.tensor_copy(
                                    hr[:, w : 2 * w], hr[:, 0:w]
                                )
                                w *= 2
                            for k2 in range(0, DIM * N_MEM, HREP_W):
                                nc.vector.tensor_mul(
                                    P_t[:, k2 : k2 + HREP_W],
                                    r_t[:, k2 : k2 + HREP_W],
                                    hr,
                                )
                        else:
                            h_b = h_sl.unsqueeze(1).broadcast_to(
                                (128, DIM, N_MEM)
                            )
                            nc.gpsimd.tensor_tensor(
                                P_t.rearrange("p (d m) -> p d m", d=DIM),
                                r_t.rearrange("p (d m) -> p d m", d=DIM),
                                h_b,
                                op=ALU.mult,
                            )
                        Pm = P_t.rearrange("p (d m) -> p m d", d=DIM)
                        for c in range(NCH):
                            nc.tensor.matmul(
                                rh_ps[c][half : half + 64, :],
                                lhsT=ones_bd,
                                rhs=Pm[:, c * MC : (c + 1) * MC, :],
                                start=(S % 2 == 0 and eg == 0),
                                stop=(S % 2 == 1 and eg == NEG - 1),
                            )

                if variant == "dma":
                    continue

                # ---- features z = [Rh*v, Rh*M, |Rh-v|, |Rh-M|] ----
                z_hop = zpool.tile([BC, N_MEM * 4 * DIM], BF16, tag="z")
                z4 = z_hop.rearrange("b (m f d) -> b m f d", f=4, d=DIM)
                vr3 = v_rep.rearrange("b (m d) -> b m d", d=DIM)
                mr3 = M_rep.rearrange("b (m d) -> b m d", d=DIM)
                for c in range(NCH):
                    mc = slice(c * MC, (c + 1) * MC)
                    rh_sb = fpool.tile([BC, MC * DIM], BF16, tag="rh_sb")
                    # Pool cannot read PSUM on HW; split these on Act/DVE
                    if c % 2 == 0:
                        nc.scalar.copy(out=rh_sb, in_=rh_ps[c])
                    else:
                        nc.vector.tensor_copy(rh_sb, rh_ps[c])
                    rh3 = rh_sb.rearrange("b (m d) -> b m d", d=DIM)
                    nc.vector.tensor_mul(z4[:, mc, 0, :], rh3, vr3)
                    nc.vector.tensor_mul(z4[:, mc, 1, :], rh3, mr3)
                    nc.gpsimd.tensor_tensor(z4[:, mc, 2, :], rh3, vr3, op=ALU.subtract)
                    nc.gpsimd.tensor_tensor(z4[:, mc, 3, :], rh3, mr3, op=ALU.subtract)
                    nc.scalar.activation(
                        z4[:, mc, 2:4, :], z4[:, mc, 2:4, :], AF.Abs
                    )

                # ---- MLP per m4-group: transpose + matmuls; groups of 4
                # stack a1 into [128, 512] (32-aligned partition offsets) so
                # one block-diag W2 matmul emits Z for 4 groups at once.
                # Final Z col-order is m' = (m4, G, g4); ts is host-permuted
                # to match (softmax is order-invariant).
                zT_ps = pp_m.tile([BC, N_MEM], F32, tag="ps1")
                zf = z_hop.rearrange("b (m fd) -> b m fd", fd=4 * DIM)
                for G in range(EV):
                    a1_4 = apool.tile([128, EV * BC], BF16, tag="a1")
                    for g4 in range(EV):
                        g = G * EV + g4
                        zt_ps = pp_zt.tile([D4, EV * BC], BF16, tag="zt")
                        for j in range(EV):
                            nc.tensor.transpose(
                                zt_ps[:, j * BC : (j + 1) * BC],
                                zf[:, g * EV + j, :],
                                ident16,
                            )
                        zt_sb = zpool.tile(
                            [D4, EV * BC], BF16, tag="zt_sb", bufs=3
                        )
                        if g % 2 == 0:
                            nc.scalar.copy(out=zt_sb, in_=zt_ps)
                        else:
                            nc.vector.tensor_copy(zt_sb, zt_ps)
                        ps1 = pp_m.tile([DIM, EV * BC], F32, tag="ps1")
                        nc.tensor.matmul(
                            ps1, lhsT=W1T, rhs=zt_sb, start=True, stop=True
                        )
                        nc.scalar.activation(
                            a1_4[g4 * DIM : (g4 + 1) * DIM, :],
                            ps1,
                            AF.Tanh,
                            bias=b1T,
                        )
                    ps2 = pp_2.tile([EV, EV * BC], F32, tag="ps2")
                    nc.tensor.matmul(ps2, lhsT=W2BD, rhs=a1_4, start=True, stop=True)
                    z4sb = zpool.tile([EV, EV * BC], F32, tag="z4sb", bufs=2)
                    if G % 2 == 0:
                        nc.scalar.copy(out=z4sb, in_=ps2)
                    else:
                        nc.vector.tensor_copy(z4sb, ps2)
                    for j in range(EV):
                        nc.tensor.transpose(
                            zT_ps[:, j * MC + G * EV : j * MC + (G + 1) * EV],
                            z4sb[:, j * BC : (j + 1) * BC],
                            ident[:EV, :EV],
                        )

                # softmax over m (skip max-subtract; |Z| tanh-bounded),
                # normalize o after the t-reduction
                e_row = small.tile([BC, N_MEM], F32, tag="e_row")
                nc.scalar.activation(e_row, zT_ps, AF.Exp)
                e16 = small.tile([BC, N_MEM], BF16, tag="e16")
                nc.scalar.copy(out=e16, in_=e_row)
                ssum = small.tile([BC, 1], F32, tag="ssum")
                nc.vector.tensor_reduce(out=ssum, in_=e_row, axis=AX.X, op=ALU.add)
                rsum = small.tile([BC, 1], F32, tag="rsum")
                nc.vector.reciprocal(rsum, ssum)

                # o[b,d] = (sum_m t[b,m,d] * e[b,m]) * rsum[b]
                t3 = t_hop.rearrange("b (m d) -> b m d", d=DIM)
                g3 = e16.unsqueeze(2).broadcast_to((BC, N_MEM, DIM))
                nc.gpsimd.tensor_tensor(t3, t3, g3, op=ALU.mult)
                for mh in (32, 16, 8, 4, 2):
                    nc.vector.tensor_add(
                        t3[:, :mh, :], t3[:, :mh, :], t3[:, mh : 2 * mh, :]
                    )
                o_raw = small.tile([BC, DIM], F32, tag="o_raw")
                nc.vector.tensor_add(
                    o_raw.unsqueeze(1), t3[:, 0:1, :], t3[:, 1:2, :]
                )
                o_row = small.tile([BC, DIM], F32, tag="o_row")
                nc.vector.tensor_scalar_mul(o_row, o_raw, rsum)

                # ---- GRU (transposed layout [*, b], f32) ----
                ot_ps = pp_m.tile([DIM, BC], F32, tag="ps1")
                nc.tensor.transpose(ot_ps, o_row, ident)
                oT = small.tile([DIM, BC], F32, tag="oT")
                nc.scalar.copy(out=oT, in_=ot_ps)

                def gate_pair(g):
                    gi = pp_m.tile([DIM, BC], F32, tag="ps1")
                    nc.tensor.matmul(
                        gi,
                        lhsT=WihT[hop][:, g * DIM : (g + 1) * DIM],
                        rhs=oT,
                        start=True,
                        stop=True,
                    )
                    gh = pp_m.tile([DIM, BC], F32, tag="ps1")
                    nc.tensor.matmul(
                        gh,
                        lhsT=WhhT[hop][:, g * DIM : (g + 1) * DIM],
                        rhs=MT,
                        start=True,
                        stop=True,
                    )
                    return gi, gh

                rz_t = []
                for g in range(2):
                    gi, gh = gate_pair(g)
                    gb = small.tile([DIM, BC], F32, tag=f"g{g}b")
                    nc.scalar.activation(gb, gi, AF.Identity, bias=bsum_rz[hop][g])
                    nc.vector.tensor_add(gb, gb, gh)
                    gt = small.tile([DIM, BC], F32, tag=f"gate{g}")
                    nc.scalar.activation(gt, gb, AF.Sigmoid)
                    rz_t.append(gt)
                r_t, z_t = rz_t

                gi_n, gh_n = gate_pair(2)
                ghn = small.tile([DIM, BC], F32, tag="ghn")
                nc.scalar.activation(ghn, gh_n, AF.Identity, bias=bhhn_t[hop])
                gin = small.tile([DIM, BC], F32, tag="gin")
                nc.scalar.activation(gin, gi_n, AF.Identity, bias=bihn_t[hop])
                n1 = small.tile([DIM, BC], F32, tag="n1")
                nc.vector.tensor_mul(n1, r_t, ghn)
                nc.vector.tensor_add(n1, n1, gin)
                n_t = small.tile([DIM, BC], F32, tag="n_t")
                nc.scalar.activation(n_t, n1, AF.Tanh)

                # M' = n + z * (M - n)
                MT_new = mstate.tile([DIM, BC], F32, tag="MT")
                nc.vector.tensor_sub(MT_new, MT, n_t)
                nc.vector.tensor_mul(MT_new, MT_new, z_t)
                nc.vector.tensor_add(MT_new, MT_new, n_t)
                MT = MT_new

                mrow_ps = pp_m.tile([BC, DIM], F32, tag="ps1")
                nc.tensor.transpose(mrow_ps, MT, ident[:DIM, :DIM])
                if hop < N_HOP - 1:
                    # M_rep [b, (m16, d)] bf16 via packed log-doubling
                    M_rep_new = mstate.tile([BC, MC * DIM], BF16, tag="M_rep")
                    nc.scalar.copy(out=M_rep_new[:, 0:DIM], in_=mrow_ps)
                    w = DIM
                    while w < MC * DIM:
                        nc.vector.tensor_copy(
                            M_rep_new[:, w : 2 * w], M_rep_new[:, 0:w]
                        )
                        w *= 2
                    M_rep = M_rep_new
                else:
                    M_row = mstate.tile([BC, DIM], F32, tag="M_row")
                    nc.scalar.copy(out=M_row, in_=mrow_ps)
                    nc.sync.dma_start(out=out_d[:, :], in_=M_row)

    nc.compile()
    return nc


_NC_CACHE = None


def _get_nc():
    global _NC_CACHE
    if _NC_CACHE is None:
        _NC_CACHE = build_nc()
    return _NC_CACHE


def _bf16(x):
    import ml_dtypes

    return np.asarray(x).astype(ml_dtypes.bfloat16)


def _fp8(x):
    import ml_dtypes

    return np.asarray(x).astype(ml_dtypes.float8_e4m3)


def permute_R(x):
    """Rs [BC, N_HOP, m, d, e] -> (Rs_v bf16, Rs_p fp8), each
    [hop, S, k, (q,e4), (d,m)]: eg = 2k + S%2 on the v side."""
    y = x.reshape(NSB, QB, N_HOP, N_MEM, DIM, NEG, EV)
    # [S, q, hop, m, d, eg, e4] -> [hop, S, eg, q, e4, d, m]
    y = y.transpose(2, 0, 5, 1, 6, 4, 3).reshape(
        N_HOP, NSB, NEG, 128, DIM * N_MEM
    )
    idx_v = np.empty((NSB, NEG // 2), dtype=np.int64)
    idx_p = np.empty((NSB, NEG // 2), dtype=np.int64)
    for S in range(NSB):
        for k in range(NEG // 2):
            idx_v[S, k] = 2 * k + S % 2
            idx_p[S, k] = 2 * k + 1 - S % 2
    rv = np.stack(
        [y[:, S, idx_v[S]] for S in range(NSB)], axis=1
    )
    rp = np.stack(
        [y[:, S, idx_p[S]] for S in range(NSB)], axis=1
    )
    return np.ascontiguousarray(_bf16(rv)), np.ascontiguousarray(_fp8(rp))


def permute_h(x):
    """hs [BC, N_HOP, m, e] -> [hop, S, (q,e4), (eg,m)] bf16."""
    y = x.reshape(NSB, QB, N_HOP, N_MEM, NEG, EV)
    # [S, q, hop, m, eg, e4] -> [hop, S, q, e4, eg, m]
    y = y.transpose(2, 0, 1, 5, 4, 3)
    return np.ascontiguousarray(y.reshape(N_HOP, NSB, 128, NEG * N_MEM))


def permute_t(x):
    """ts [BC, hop, m, d]: m reordered to m' = (m4, G, g4), m = (4G+g4)*4+m4."""
    y = x.reshape(BC, N_HOP, EV, EV, EV, DIM)  # [b, hop, G, g4, m4, d]
    return np.ascontiguousarray(
        y.transpose(0, 1, 4, 2, 3, 5).reshape(BC, N_HOP, N_MEM, DIM)
    )


def make_in_maps(hs, Rs, ts, vs, W1, b1, W2, W_ih, W_hh, b_ih, b_hh):
    in_maps = []
    for c in range(N_CORES):
        sl = slice(c * BC, (c + 1) * BC)
        rv, rp = permute_R(Rs[sl])
        in_maps.append(
            {
                "Rs_v": rv,
                "Rs_p": rp,
                "hs": permute_h(_bf16(hs[sl])),
                "ts": permute_t(_bf16(ts[sl])),
                "vs": np.ascontiguousarray(vs[sl]),
                "W1": np.ascontiguousarray(W1),
                "b1": np.ascontiguousarray(b1),
                "W2": np.ascontiguousarray(W2),
                "W_ih": np.ascontiguousarray(W_ih),
                "W_hh": np.ascontiguousarray(W_hh),
                "b_ih": np.ascontiguousarray(b_ih),
                "b_hh": np.ascontiguousarray(b_hh),
            }
        )
    return in_maps


def kernel(hs, Rs, ts, vs, W1, b1, W2, b2, W_ih, W_hh, b_ih, b_hh):
    from concourse.bass_utils import run_bass_kernel_spmd

    nc = _get_nc()
    in_maps = make_in_maps(hs, Rs, ts, vs, W1, b1, W2, W_ih, W_hh, b_ih, b_hh)
    res = run_bass_kernel_spmd(nc, in_maps, list(range(N_CORES)))
    return np.concatenate([r["out"] for r in res.results], axis=0)
